# revision 37
# baseline (speedup 1.0000x reference)
"""Trainium2 Bass kernel for per-pixel dot-product attention.

Reference op (per pixel, over C=80 channels split q/k/v = 8/64/8):
    qk[v] = sum_k q[k] * K[k, v] / sqrt(8)
    attn  = softmax(qk over v)
    out[v] = attn[v] * V[v]

Strategy: pure data-parallel over 8 NeuronCores — core i handles batch
i//2, H-rows half (i%2).  The per-core shard is pre-transposed on the
HOST to a partition-major layout [128, C * 1024]: partition p owns
pixels [p*1024, (p+1)*1024), and the free dim is a concatenation of
per-chunk [C x ncol] channel-major blocks.  Each chunk then loads with
ONE HWDGE dma_start whose descriptors are C*ncol*4 (~50 KB) contiguous
bytes per partition — per-SDMA-engine cost is ~12ns + bytes/27GB/s, so
big descriptors reach ~420 GB/s aggregate vs 339 for the 1 KB
descriptors a pixel-major layout forces.  ScalarE converts q and the 8
K pieces to bf16; DVE multiplies each piece and serially accumulates
(acc += piece), so qk is ready one add after the last multiply.  Chunk
j's softmax (exp on ACT; bf16 v-tree, f32 sum, reciprocal, two output
multiplies on DVE) is emitted one piece into chunk j+1's product
stream so neither engine idles at the chunk boundary.  The final
multiply writes a bf16 out tile stored to a bf16 y (host upcasts),
halving output HBM bytes; rel-l2 err ~5e-3 vs the 2e-2 gate.

Timing on trn2 (8 cores, NTFF): 145.7 us/NEFF (best; ~156 us when the
shared HBM is externally contended), vs 161.1 us for the previous
pixel-major kernel.  Steady state is a coupled equilibrium:
per-160-col chunk the DMA needs ~15.5 us and DVE ~16.9 us, but when
the DMA streams back-to-back all engine ops slow ~20% (SBUF port
contention), so the good regime keeps the DMA slightly compute-gated
(~92% duty).  Schedules that let the DMA run continuously (small first
chunk, SWDGE cast-loads, leaner softmax) all measured SLOWER — see
NOTES.md for the full experiment log.
"""

import numpy as np

NK = 8
NV = 8
C = NK + NK * NV + NV  # 80
B, H, W = 4, 512, 512
N_CORES = 8
ROWS = H // 2            # rows per core
PIX = ROWS * W           # pixels per core (131072)
XCOLS = PIX // 128       # free-dim pixels per partition (1024)
_SCALE = 1.0 / float(np.sqrt(NK))

# per-chunk free-dim widths; big head chunks for DMA efficiency, tapered
# tail so the post-prod serial chain (softmax/out) drains fast
CHUNKS = [160, 160, 160, 160, 160, 128, 48, 32, 16]
assert sum(CHUNKS) == XCOLS


def _ensure_path():
    import sys
    p = "/opt/trn_rl_repo"
    if p not in sys.path:
        sys.path.insert(0, p)


def build_nc(chunk_cols=None, in_bufs=3, e_bufs=2, o_bufs=2,
             acc_bufs=2, piece_bufs=3, recip_on_act=False, swdge_cast=False,
             vr_bf16=True, sm_after=1, split_first=False, pair_mults=False,
             store_on_sync=True):
    """Per-core Bass program: x [128, C*XCOLS] f32 -> y [128, NV*XCOLS] bf16.

    One input dma_start per chunk (sync ring), one bf16 output store per
    chunk (scalar ring, deferred behind the next chunk's load trigger).
    The k-reduction is a serial accumulator — each converted K piece is
    multiplied by q and immediately added into acc — so qk (= acc) is
    ready one add after the last multiply and the inline exp stalls ACT
    only ~1 us.  Rotating piece buffers keep the conversions streaming.
    """
    _ensure_path()
    import concourse.tile as tile
    from concourse import bacc, mybir

    f32 = mybir.dt.float32
    bf16 = mybir.dt.bfloat16
    if chunk_cols is None:
        chunk_cols = CHUNKS
    assert sum(chunk_cols) == XCOLS

    nc = bacc.Bacc("TRN2", target_bir_lowering=False, debug=False)
    x = nc.dram_tensor("x", [128, C * XCOLS], f32, kind="ExternalInput")
    y = nc.dram_tensor("y", [128, NV * XCOLS], bf16, kind="ExternalOutput")

    pending_out = []

    def flush_out():
        ring = nc.sync if store_on_sync else nc.scalar
        for args in pending_out:
            ring.dma_start(**args)
        pending_out.clear()

    with tile.TileContext(nc) as tc:
        with (
            tc.tile_pool(name="inp", bufs=1) as in_pool,
            tc.tile_pool(name="work", bufs=1) as work_pool,
            tc.tile_pool(name="pipe", bufs=1) as pipe_pool,
        ):
            def emit_softmax(st):
                """exp + v-sum + reciprocal + output multiplies for a chunk
                whose accumulator is complete.  Called one piece into the
                NEXT chunk's product stream so the exp overlaps DVE's first
                multiply and the DVE ops interleave with later multiplies."""
                j, n, off, vsrc, acc = st
                e = pipe_pool.tile([128, NV * n], bf16, name=f"e{j}", tag="e",
                                   bufs=e_bufs)
                nc.scalar.activation(e, acc, mybir.ActivationFunctionType.Exp,
                                     scale=_SCALE)
                t1 = pipe_pool.tile([128, 4 * n], bf16, name=f"t1_{j}",
                                    tag="t1", bufs=1)
                nc.vector.tensor_tensor(t1, e[:, 0:4 * n], e[:, 4 * n:],
                                        mybir.AluOpType.add)
                nc.vector.tensor_tensor(t1[:, 0:2 * n], t1[:, 0:2 * n],
                                        t1[:, 2 * n:], mybir.AluOpType.add)
                sc = pipe_pool.tile([128, 2 * n], f32, name=f"sc{j}",
                                    tag="sc", bufs=1)
                s = sc[:, 0:n]
                nc.vector.tensor_tensor(s, t1[:, 0:n], t1[:, n:2 * n],
                                        mybir.AluOpType.add)
                # out[v] = e[v] * (V[v] * (1/s)); with vr_bf16 the recip
                # writes bf16 and multiplies an ACT-converted bf16 v at 2x
                # rate.  Stride-0 broadcast operand goes in in0.
                vr = pipe_pool.tile([128, NV * n], bf16, name=f"vr{j}",
                                    tag="vr", bufs=1)
                vr3 = vr.rearrange("p (v x) -> p v x", v=NV)
                if vr_bf16:
                    rb16 = pipe_pool.tile([128, n], bf16, name=f"r{j}",
                                          tag="r", bufs=1)
                    with nc.allow_low_precision(reason="softmax weight in bf16"):
                        nc.vector.reciprocal(rb16, s)
                    r_b = rb16.unsqueeze(1).broadcast_to((128, NV, n))
                else:
                    r = sc[:, n:2 * n]
                    nc.vector.reciprocal(r, s)
                    r_b = r.unsqueeze(1).broadcast_to((128, NV, n))
                nc.vector.tensor_tensor(vr3, r_b, vsrc, mybir.AluOpType.mult)
                ob = pipe_pool.tile([128, NV * n], bf16, name=f"o{j}",
                                    tag="o", bufs=o_bufs)
                nc.vector.tensor_tensor(ob, vr, e, mybir.AluOpType.mult)
                pending_out.append(dict(
                    out=y[:, NV * off:NV * (off + n)], in_=ob,
                ))

            pending_sm = None
            off = 0
            for j, n in enumerate(chunk_cols):
                it = in_pool.tile([128, C * n], f32, name=f"in{j}", tag="in",
                                  bufs=in_bufs)
                if j == 0 and split_first:
                    # ramp: chunk 0's load in channel-range pieces so its
                    # conversions start when q + the first K piece land
                    for a, b in ((0, 16), (16, 40), (40, C)):
                        nc.sync.dma_start(
                            out=it[:, a * n:b * n],
                            in_=x[:, C * off + a * n:C * off + b * n])
                else:
                    nc.sync.dma_start(out=it,
                                      in_=x[:, C * off:C * (off + n)])
                # chunk j-2's store, behind this chunk's load trigger
                flush_out()

                q_bf = work_pool.tile([128, NK * n], bf16, name=f"qbf{j}",
                                      tag="qbf", bufs=2)
                nc.scalar.activation(q_bf, it[:, 0:NK * n],
                                     mybir.ActivationFunctionType.Copy)
                q_b = (
                    q_bf.rearrange("p (k x) -> p k x", k=NK)
                    .unsqueeze(2)
                    .broadcast_to((128, NK, NV, n))
                )

                # serial-accumulated products: acc = sum_k q[k] * K[k, :]
                acc = work_pool.tile([128, NV * n], bf16, name=f"acc{j}",
                                     tag="acc", bufs=acc_bufs)
                a4 = acc.rearrange("p (v x) -> p v x", v=NV).unsqueeze(1)
                if pair_mults:
                    # 4 two-piece conversions/multiplies (fewer op
                    # boundaries); adds stay 7 x 8-block
                    for h in range(NK // 2):
                        pc2 = work_pool.tile([128, 2 * NV * n], bf16,
                                             name=f"pc{j}_{h}", tag="pc",
                                             bufs=piece_bufs)
                        p4 = pc2.rearrange("p (k v x) -> p k v x", k=2, v=NV)
                        nc.scalar.activation(
                            pc2,
                            it[:, (NK + 2 * h * NV) * n:
                               (NK + (2 * h + 2) * NV) * n],
                            mybir.ActivationFunctionType.Copy)
                        nc.vector.tensor_tensor(
                            p4, q_b[:, 2 * h:2 * h + 2], p4,
                            mybir.AluOpType.mult)
                        if h == 0:
                            nc.vector.tensor_tensor(
                                acc, pc2[:, 0:NV * n], pc2[:, NV * n:],
                                mybir.AluOpType.add)
                            if pending_sm is not None:
                                emit_softmax(pending_sm)
                                pending_sm = None
                            if vr_bf16:
                                vb = work_pool.tile(
                                    [128, NV * n], bf16, name=f"vb{j}",
                                    tag="vb", bufs=2)
                                nc.scalar.activation(
                                    vb, it[:, (NK + NK * NV) * n:C * n],
                                    mybir.ActivationFunctionType.Copy)
                        else:
                            nc.vector.tensor_tensor(
                                acc, acc, pc2[:, 0:NV * n],
                                mybir.AluOpType.add)
                            nc.vector.tensor_tensor(
                                acc, acc, pc2[:, NV * n:],
                                mybir.AluOpType.add)
                    vsrc = (vb.rearrange("p (v x) -> p v x", v=NV)
                            if vr_bf16 else
                            it.rearrange("p (c x) -> p c x", c=C)
                            [:, NK + NK * NV:C])
                    pending_sm = (j, n, off, vsrc, acc)
                    off += n
                    continue
                for k in range(NK):
                    src = it[:, (NK + k * NV) * n:(NK + (k + 1) * NV) * n]
                    if k == 0:
                        nc.scalar.activation(
                            acc, src, mybir.ActivationFunctionType.Copy)
                        nc.vector.tensor_tensor(
                            a4, q_b[:, 0:1], a4, mybir.AluOpType.mult)
                    else:
                        pc = work_pool.tile([128, NV * n], bf16,
                                            name=f"pc{j}_{k}", tag="pc",
                                            bufs=piece_bufs)
                        p4 = pc.rearrange("p (v x) -> p v x", v=NV)\
                            .unsqueeze(1)
                        nc.scalar.activation(
                            pc, src, mybir.ActivationFunctionType.Copy)
                        nc.vector.tensor_tensor(
                            p4, q_b[:, k:k + 1], p4, mybir.AluOpType.mult)
                        nc.vector.tensor_tensor(acc, acc, pc,
                                                mybir.AluOpType.add)
                    # previous chunk's softmax rides a few pieces into this
                    # chunk's product stream: its exp runs on ACT while DVE
                    # does the first multiplies, and its DVE tail
                    # interleaves with the later pieces
                    if k == sm_after and pending_sm is not None:
                        emit_softmax(pending_sm)
                        pending_sm = None
                    if k == sm_after and vr_bf16:
                        # bf16 copy of v for this chunk's own softmax
                        vb = work_pool.tile([128, NV * n], bf16,
                                            name=f"vb{j}", tag="vb", bufs=2)
                        nc.scalar.activation(
                            vb, it[:, (NK + NK * NV) * n:C * n],
                            mybir.ActivationFunctionType.Copy)

                vsrc = vb.rearrange("p (v x) -> p v x", v=NV) if vr_bf16 else \
                    it.rearrange("p (c x) -> p c x", c=C)[:, NK + NK * NV:C]
                pending_sm = (j, n, off, vsrc, acc)
                off += n
            emit_softmax(pending_sm)
            flush_out()
    nc.compile()
    return nc


_NC_CACHE = {}

BUILD_CFG = {}


def _get_nc(**cfg):
    cfg = {**BUILD_CFG, **cfg}
    key = tuple(sorted(
        (k, tuple(v) if isinstance(v, list) else v) for k, v in cfg.items()
    ))
    if key not in _NC_CACHE:
        _NC_CACHE[key] = build_nc(**cfg)
    return _NC_CACHE[key]


def make_in_maps(inp, chunk_cols=None):
    """Host-side shard + transpose to the partition-major chunked layout."""
    if chunk_cols is None:
        chunk_cols = CHUNKS
    in_maps = []
    for core in range(N_CORES):
        b, hh = core // 2, core % 2
        t3 = np.asarray(
            inp[b, :, hh * ROWS:(hh + 1) * ROWS, :], dtype=np.float32
        ).reshape(C, 128, XCOLS).transpose(1, 0, 2)  # [128, C, XCOLS]
        off = 0
        parts = []
        for n in chunk_cols:
            parts.append(np.ascontiguousarray(
                t3[:, :, off:off + n]).reshape(128, C * n))
            off += n
        in_maps.append({"x": np.ascontiguousarray(
            np.concatenate(parts, axis=1))})
    return in_maps


def assemble_out(results, chunk_cols=None):
    if chunk_cols is None:
        chunk_cols = CHUNKS
    out = np.empty((B, NV, H, W), np.float32)
    for core in range(N_CORES):
        b, hh = core // 2, core % 2
        r = np.asarray(results[core]["y"]).astype(np.float32)  # [128, NV*XCOLS]
        off = 0
        blocks = []
        for n in chunk_cols:
            blocks.append(r[:, NV * off:NV * (off + n)].reshape(128, NV, n))
            off += n
        img = np.concatenate(blocks, axis=2)          # [128, NV, XCOLS]
        out[b, :, hh * ROWS:(hh + 1) * ROWS, :] = (
            img.transpose(1, 0, 2).reshape(NV, ROWS, W)
        )
    return out


def run_spmd(inp, trace=False, build_cfg=None, **kwargs):
    """Run the SPMD kernel on 8 cores; returns (full_output, BassKernelResults)."""
    _ensure_path()
    from concourse.bass_utils import run_bass_kernel_spmd

    inp = np.asarray(inp)
    assert inp.shape == (B, C, H, W), inp.shape
    cfg = dict(build_cfg or {})
    chunk_cols = cfg.get("chunk_cols") or CHUNKS
    nc = _get_nc(**cfg)
    res = run_bass_kernel_spmd(
        nc, make_in_maps(inp, chunk_cols), list(range(N_CORES)),
        trace=trace, **kwargs
    )
    return assemble_out(res.results, chunk_cols), res


def kernel(inp):
    out, _ = run_spmd(inp, trace=False)
    return out
